# revision 27
# baseline (speedup 1.0000x reference)
"""Bass/Tile TRN2 kernel for nn_MultiHeadAttention_56066503082210.

Full-input contract: kernel(**inputs) takes the complete tensors and returns
the complete [B, N, D] output. Internally shards batch across 8 NeuronCores
(data parallel, no collectives) and runs one SPMD Bass program per core.

Per-core pipeline (batch b):
  x.T via PE transposes -> q/k/v projections (fp32r matmuls)
  RMSNorm + 2D RoPE on q/k in natural [n, dout] layout (gamma & rope-validity
  folded into host-precomputed cos/sin tables), then PE-transpose to [hd, n]
  S.T = k.T^T q.T per head -> tanh softcap (ACT) -> exp (ACT) -> mask multiply
  P.T @ [V | ones] accumulates attention output AND softmax denominators
  normalize, then final projection with Wo.
"""

import sys

for p in ("/opt/trn_rl_repo", "/root/.axon_site/_ro/trn_rl_repo"):
    if p not in sys.path:
        sys.path.insert(0, p)

import numpy as np
import ml_dtypes

import concourse.bass as bass
import concourse.mybir as mybir
import concourse.tile as tile
from concourse.tile import TileContext
from concourse.masks import make_identity
from concourse.bass_utils import run_bass_kernel_spmd

# ---------------------------------------------------------------- constants
B, N, D, H, HD = 8, 1024, 1024, 16, 64
NT = N // 128          # n tiles
KT = D // 128          # contraction chunks
SOFT_CAP = 50.0
EPS = 1e-6
SCALE = HD ** -0.5     # 1/8
N_ONES = 32            # replicated ones columns in V_aug (denominator rows)
VCOLS = HD + N_ONES    # 96
F32 = mybir.dt.float32
F32R = mybir.dt.float32r
BF16 = mybir.dt.bfloat16
TANH = mybir.ActivationFunctionType.Tanh
EXP = mybir.ActivationFunctionType.Exp
SQRT = mybir.ActivationFunctionType.Sqrt
ADD = mybir.AluOpType.add
MULT = mybir.AluOpType.mult

# ------------------------------------------------- walrus compat monkeypatches
# This walrus build accepts at most ONE semaphore wait per instruction for
# several instruction types (fp32r Matmult, Drain, ...). Split excess waits
# onto injected same-engine NoOps, which execute the waits in program order.
_PATCHED = False


def _apply_patches():
    global _PATCHED
    if _PATCHED:
        return
    _PATCHED = True

    _orig_lower = TileContext._lower_ordered_insts

    def _split_waits(self, ordered):
        counter = [0]
        for bb_name, insts in ordered.items():
            out = []
            for inst in insts:
                si = inst.sync_info
                waits = list(si.on_wait or []) if si is not None else []
                if len(waits) > 1:
                    for w in waits[:-1]:
                        counter[0] += 1
                        nop = mybir.InstNoOp(
                            name=f"I-waitsplit-{bb_name}-{counter[0]}",
                            engine=inst.engine,
                            ins=[],
                            outs=[],
                            sync_info=mybir.SyncInfo(on_wait=[w], on_update=[]),
                        )
                        out.append(nop)
                    si.on_wait = waits[-1:]
                out.append(inst)
            insts[:] = out
        return _orig_lower(self, ordered)

    TileContext._lower_ordered_insts = _split_waits

    def _patched_drain(self, tick_clock, wait_clock):
        nc = self.nc
        drain_inst = nc.sync.drain()
        wait_clock.add_sem_waits(
            drain_inst.ins, tile.ScopedClock({None: tick_clock.global_clock})
        )
        si = drain_inst.ins.sync_info
        waits = list(si.on_wait or []) if si is not None else []
        if len(waits) > 1:
            si.on_wait = waits[:1]
            for w in waits[1:]:
                n = nc.sync.nop(nofuse=True, hint="tail_wait_split")
                n.ins.sync_info = mybir.SyncInfo(on_wait=[w], on_update=[])
            nc.sync.drain()
        nc.all_engine_barrier()
        assert self.sems is not None
        popped = nc._tile_sem_poison_stack.pop()
        assert popped is self._sem_poison
        nc.clear_and_free_semaphores(list(self.sems.allocated().values()))
        nc.all_engine_barrier()

    TileContext._drain_and_barrier = _patched_drain

    # Enable walrus LDWEIGHTS dedup (adjacent matmuls sharing a stationary
    # operand skip the redundant reload). Correctness is revalidated by the
    # accuracy check after every change.
    import concourse.bass_utils as _bu

    _orig_run_command = _bu.run_command

    def _run_command(cmd, *a, **kw):
        if isinstance(cmd, list):
            pass  # ldw-opt stays disabled: walrus emits wrong-length ISA with it on
        return _orig_run_command(cmd, *a, **kw)

    _bu.run_command = _run_command


# ------------------------------------------------------------- device program
def build_program():
    _apply_patches()
    nc = bass.Bass()

    x_d = nc.dram_tensor("x", [N, D], F32, kind="ExternalInput")
    wq_d = nc.dram_tensor("wqT", [D, D], F32R, kind="ExternalInput")
    wk_d = nc.dram_tensor("wkT", [D, D], F32R, kind="ExternalInput")
    wv_d = nc.dram_tensor("wvT", [D, D], F32R, kind="ExternalInput")
    wo_d = nc.dram_tensor("woT", [D, D], F32R, kind="ExternalInput")
    cosq_d = nc.dram_tensor("cosq", [N, HD], F32, kind="ExternalInput")
    sinq_d = nc.dram_tensor("sinq", [N, HD], F32, kind="ExternalInput")
    cosk_d = nc.dram_tensor("cosk", [N, HD], F32, kind="ExternalInput")
    sink_d = nc.dram_tensor("sink", [N, HD], F32, kind="ExternalInput")
    mask_d = nc.dram_tensor("mask01T", [N, N], BF16, kind="ExternalInput")
    out_d = nc.dram_tensor("out", [N, D], F32, kind="ExternalOutput")

    with TileContext(nc) as tc:
        with (
            tc.tile_pool(name="pa", bufs=1) as pa,
            tc.tile_pool(name="pqk", bufs=1) as pqk,
        ):
            ident = pa.tile([128, 128], F32)
            make_identity(nc, ident[:])
            eps_b = pa.tile([128, 1], F32)
            nc.vector.memset(eps_b[:], EPS)

            # rope tables, [p, nt, j] layout
            tabs = {}
            for name, d in (("cosq", cosq_d), ("sinq", sinq_d),
                            ("cosk", cosk_d), ("sink", sink_d)):
                t = pa.tile([128, NT, HD], F32, tag=name)
                nc.sync.dma_start(t[:], d.rearrange("(t p) j -> p t j", p=128))
                tabs[name] = t

            # V_aug [p, h, c, col]: col<HD = v values, col>=HD = 1.0
            vaug = pa.tile([128, H, KT, VCOLS], F32R)
            one_c = pa.tile([128, 1], F32)
            nc.vector.memset(one_c[:], 1.0)
            nc.vector.tensor_copy(
                vaug[:, :, :, HD:VCOLS],
                one_c[:, None, None, :].broadcast_to([128, H, KT, N_ONES]))

            # persistent transposed q/k; qnT is later overwritten per-head with
            # the normalized attention output O.T (same lifetime handoff)
            qnT = pqk.tile([128, KT, N], F32R)
            knT = pqk.tile([128, KT, N], F32R)

            with (
                tc.tile_pool(name="pxs", bufs=1) as pxs,
                tc.tile_pool(name="px", bufs=2) as px,
                tc.tile_pool(name="ps_tp", bufs=2, space="PSUM") as ps_tp,
                tc.tile_pool(name="ps_mm", bufs=2, space="PSUM") as ps_mm,
            ):
                # ---- phase 0: load x, build x.T
                xT = pxs.tile([128, KT, N], F32R)
                for nt in range(NT):
                    xin = px.tile([128, D], F32, tag="xin")
                    nc.sync.dma_start(xin[:], x_d[nt * 128:(nt + 1) * 128, :])
                    for kt in range(KT):
                        tp = ps_tp.tile([128, 128], F32, tag="tp")
                        nc.tensor.transpose(tp[:], xin[:, kt * 128:(kt + 1) * 128], ident[:])
                        nc.vector.tensor_copy(xT[:, kt, nt * 128:(nt + 1) * 128], tp[:])

                # ---- phase 1: projections (whole W resident per projection)
                def load_w_all(dram):
                    w = pxs.tile([128, KT, D], F32R, tag="wall")
                    for kt in range(KT):
                        nc.sync.dma_start(
                            w[:, kt, :], dram[kt * 128:(kt + 1) * 128, :])
                    return w

                # q / k with norm + rope, written transposed
                for kind in ("q", "k"):
                    w = load_w_all(wq_d if kind == "q" else wk_d)
                    cos_t = tabs["cosq" if kind == "q" else "cosk"]
                    sin_t = tabs["sinq" if kind == "q" else "sink"]
                    dst = qnT if kind == "q" else knT
                    for nt in range(NT):
                        # both halves accumulate together so adjacent matmuls
                        # share the stationary xT chunk (LDW dedup)
                        accs = [ps_mm.tile([128, 512], F32, tag=f"acc{dh}", name=f"acc{dh}")
                                for dh in range(2)]
                        for kt in range(KT):
                            for dh in range(2):
                                nc.tensor.matmul(
                                    accs[dh][:],
                                    xT[:, kt, nt * 128:(nt + 1) * 128],
                                    w[:, kt, dh * 512:(dh + 1) * 512],
                                    start=(kt == 0), stop=(kt == KT - 1),
                                )
                        for dh in range(2):
                            acc = accs[dh]
                            # copy projection to SBUF (DVE reads 1 PSUM max)
                            qc = px.tile([128, 8, HD], F32, tag="qc")
                            nc.vector.tensor_copy(
                                qc[:], acc[:].rearrange("p (g j) -> p g j", g=8))
                            a3 = qc[:]
                            # sum of squares per 64-wide head group
                            sq = px.tile([128, 8, HD], F32, tag="sq")
                            nc.vector.tensor_mul(sq[:], a3, a3)
                            ssq = px.tile([128, 8], F32, tag="ssq")
                            nc.vector.tensor_reduce(
                                ssq[:], sq[:], axis=mybir.AxisListType.X, op=ADD)
                            rstd = px.tile([128, 8], F32, tag="rstd")
                            nc.scalar.activation(
                                rstd[:], ssq[:], SQRT, bias=eps_b[:], scale=1.0 / HD)
                            nc.vector.reciprocal(rstd[:], rstd[:])
                            # normalize in place
                            nc.vector.tensor_mul(
                                qc[:], a3,
                                rstd[:, :, None].broadcast_to([128, 8, HD]))
                            # rope (gamma and validity folded into tables)
                            qr = px.tile([128, 8, HD], F32, tag="sq")
                            tmp = px.tile([128, 8, 32], F32, tag="tmp")
                            c_lo = cos_t[:, nt, 0:32][:, None, :].broadcast_to([128, 8, 32])
                            c_hi = cos_t[:, nt, 32:64][:, None, :].broadcast_to([128, 8, 32])
                            s_lo = sin_t[:, nt, 0:32][:, None, :].broadcast_to([128, 8, 32])
                            s_hi = sin_t[:, nt, 32:64][:, None, :].broadcast_to([128, 8, 32])
                            nc.gpsimd.tensor_mul(qr[:, :, 0:32], qc[:, :, 0:32], c_lo)
                            nc.gpsimd.tensor_mul(tmp[:], qc[:, :, 32:64], s_lo)
                            nc.gpsimd.tensor_add(qr[:, :, 0:32], qr[:, :, 0:32], tmp[:])
                            nc.gpsimd.tensor_mul(qr[:, :, 32:64], qc[:, :, 32:64], c_hi)
                            nc.gpsimd.tensor_mul(tmp[:], qc[:, :, 0:32], s_hi)
                            nc.gpsimd.tensor_add(qr[:, :, 32:64], qr[:, :, 32:64], tmp[:])
                            # transpose 4 blocks of 128 into dst
                            qr2 = qr[:].rearrange("p g j -> p (g j)")
                            for j in range(4):
                                dt = dh * 4 + j
                                tp = ps_tp.tile([128, 128], F32, tag="tp")
                                nc.tensor.transpose(
                                    tp[:], qr2[:, j * 128:(j + 1) * 128], ident[:])
                                nc.vector.tensor_copy(
                                    dst[:, dt, nt * 128:(nt + 1) * 128], tp[:])

                # v: natural layout straight into V_aug
                w = load_w_all(wv_d)
                for nt in range(NT):
                    accs = [ps_mm.tile([128, 512], F32, tag=f"acc{dh}", name=f"acc{dh}")
                            for dh in range(2)]
                    for kt in range(KT):
                        for dh in range(2):
                            nc.tensor.matmul(
                                accs[dh][:],
                                xT[:, kt, nt * 128:(nt + 1) * 128],
                                w[:, kt, dh * 512:(dh + 1) * 512],
                                start=(kt == 0), stop=(kt == KT - 1),
                            )
                    for dh in range(2):
                        nc.vector.tensor_copy(
                            vaug[:, dh * 8:(dh + 1) * 8, nt, 0:HD],
                            accs[dh][:].rearrange("p (g j) -> p g j", g=8)[:, :, None, :],
                        )

            # ---- phase 2: attention per head
            with (
                tc.tile_pool(name="pls", bufs=1) as pls,
                tc.tile_pool(name="pl", bufs=2) as pl,
            ):
                woT = pls.tile([128, KT, D], F32R)
                for kt in range(KT):
                    nc.sync.dma_start(
                        woT[:, kt, :], wo_d[kt * 128:(kt + 1) * 128, :])

                # mask01 transposed: [p, c, nq] (bf16 0/1)
                maskm = pls.tile([128, NT, N], BF16)
                nc.sync.dma_start(
                    maskm[:], mask_d.rearrange("(c p) q -> p c q", p=128))

                with (
                    tc.tile_pool(name="ps_sa", bufs=1, space="PSUM") as ps_sa,
                    tc.tile_pool(name="ps_sb", bufs=1, space="PSUM") as ps_sb,
                    tc.tile_pool(name="ps_pv", bufs=2, space="PSUM") as ps_pv,
                ):
                    for g in range(H // 2):
                        heads = (2 * g, 2 * g + 1)
                        spool = {2 * g: ps_sa, 2 * g + 1: ps_sb}
                        pvs = {}
                        for h in heads:
                            pvs[h] = ps_pv.tile([VCOLS, N], F32, tag="pv",
                                                name=f"pv{h}")
                        for c in range(KT):
                            pms = {}
                            for h in heads:
                                r0 = 64 * (h % 2)
                                dt = h // 2
                                qh = qnT[r0:r0 + 64, dt, :]
                                kh = knT[r0:r0 + 64, dt, :]
                                s1 = spool[h].tile([128, N], F32, tag="s1",
                                                   name=f"s1_{h}_{c}")
                                for half in range(2):
                                    nc.tensor.matmul(
                                        s1[:, half * 512:(half + 1) * 512],
                                        kh[:, c * 128:(c + 1) * 128],
                                        qh[:, half * 512:(half + 1) * 512],
                                        start=True, stop=True,
                                    )
                                t = pl.tile([128, N], F32, tag="t")
                                nc.scalar.activation(t[:], s1[:], TANH,
                                                     scale=SCALE / SOFT_CAP)
                                e = pl.tile([128, N], F32, tag="e")
                                nc.scalar.activation(e[:], t[:], EXP,
                                                     scale=SOFT_CAP)
                                pm = pl.tile([128, N], F32R, tag="pm")
                                nc.gpsimd.tensor_mul(pm[:], e[:],
                                                     maskm[:, c, :])
                                pms[h] = pm
                            for h in heads:
                                for half in range(2):
                                    nc.tensor.matmul(
                                        pvs[h][:, half * 512:(half + 1) * 512],
                                        vaug[:, h, c, :],
                                        pms[h][:, half * 512:(half + 1) * 512],
                                        start=(c == 0), stop=(c == KT - 1),
                                    )
                        for h in heads:
                            r0 = 64 * (h % 2)
                            dt = h // 2
                            pv = pvs[h]
                            recip = pl.tile([N_ONES, N], F32, tag="recip")
                            nc.vector.reciprocal(recip[:], pv[HD:VCOLS, :])
                            # normalized O.T into qnT storage (q rows dead)
                            for i in range(HD // N_ONES):
                                nc.vector.tensor_mul(
                                    qnT[r0 + i * N_ONES:r0 + (i + 1) * N_ONES,
                                        dt, :],
                                    pv[i * N_ONES:(i + 1) * N_ONES, :],
                                    recip[:],
                                )

                # ---- phase 3: output projection
                with tc.tile_pool(name="ps_o", bufs=2, space="PSUM") as ps_o:
                    for nt in range(NT):
                        accs = [ps_o.tile([128, 512], F32, tag=f"oacc{dh}", name=f"oacc{dh}")
                                for dh in range(2)]
                        for kt in range(KT):
                            for dh in range(2):
                                nc.tensor.matmul(
                                    accs[dh][:],
                                    qnT[:, kt, nt * 128:(nt + 1) * 128],
                                    woT[:, kt, dh * 512:(dh + 1) * 512],
                                    start=(kt == 0), stop=(kt == KT - 1),
                                )
                        for dh in range(2):
                            osb = pl.tile([128, 512], F32, tag="osb")
                            nc.vector.tensor_copy(osb[:], accs[dh][:])
                            nc.sync.dma_start(
                                out_d[nt * 128:(nt + 1) * 128,
                                      dh * 512:(dh + 1) * 512], osb[:])
    return nc


_NC_CACHE = None


def _get_program():
    global _NC_CACHE
    if _NC_CACHE is None:
        _NC_CACHE = build_program()
    return _NC_CACHE


# ------------------------------------------------------------------ host side
def _host_prep(Wq, Wk, Wv, Wo, q_gamma, k_gamma, cos, sin, rope_indices, mask):
    f = np.float32
    wqT = np.ascontiguousarray(np.asarray(Wq, f).T)
    wkT = np.ascontiguousarray(np.asarray(Wk, f).T)
    wvT = np.ascontiguousarray(np.asarray(Wv, f).T)
    woT = np.ascontiguousarray(np.asarray(Wo, f).T)

    idx = np.asarray(rope_indices)
    valid = (idx >= 0)
    safe = np.clip(idx, 0, None).astype(np.int64)
    cos_sel = np.asarray(cos, f)[safe]          # [N, HD]
    sin_sel = np.asarray(sin, f)[safe]
    cos_eff = np.where(valid[:, None], cos_sel, f(1.0))
    sin_eff = np.where(valid[:, None], sin_sel, f(0.0))
    # rotate_half sign: -sin on first half, +sin on second
    sin_signed = np.concatenate([-sin_eff[:, :32], sin_eff[:, 32:]], axis=1)
    gq = np.asarray(q_gamma, f)
    gk = np.asarray(k_gamma, f)
    gq_swap = np.concatenate([gq[32:], gq[:32]])
    gk_swap = np.concatenate([gk[32:], gk[:32]])
    cosq = np.ascontiguousarray(cos_eff * gq[None, :])
    sinq = np.ascontiguousarray(sin_signed * gq_swap[None, :])
    cosk = np.ascontiguousarray(cos_eff * gk[None, :])
    sink = np.ascontiguousarray(sin_signed * gk_swap[None, :])

    m01T = np.ascontiguousarray(
        np.asarray(mask).astype(np.float32).T.astype(ml_dtypes.bfloat16))
    return dict(wqT=wqT, wkT=wkT, wvT=wvT, woT=woT,
                cosq=cosq, sinq=sinq, cosk=cosk, sink=sink, mask01T=m01T)


def _ensure_profile_hook():
    """Register the NTFF profile hook (missing antenv.axon_hooks shim)."""
    import types

    try:
        from antenv.axon_hooks import get_axon_ntff_profile_hook
        if get_axon_ntff_profile_hook() is not None:
            return
        import antenv.axon_hooks as mod
    except ImportError:
        import antenv
        mod = types.ModuleType("antenv.axon_hooks")
        holder = {}
        mod.set_axon_ntff_profile_hook = lambda h: holder.__setitem__("h", h)
        mod.get_axon_ntff_profile_hook = lambda: holder.get("h")
        sys.modules["antenv.axon_hooks"] = mod
        antenv.axon_hooks = mod
    if "/root/.axon_site" not in sys.path:
        sys.path.insert(0, "/root/.axon_site")
    from trn_agent_boot.trn_boot import _ntff_profile_via_ctypes
    hook = _ntff_profile_via_ctypes("/opt/axon/libaxon_pjrt.so")
    if hook is not None:
        mod.set_axon_ntff_profile_hook(hook)


def kernel(x, Wq, Wk, Wv, Wo, q_gamma, k_gamma, cos, sin, rope_indices, mask,
           _trace=False):
    if _trace:
        _ensure_profile_hook()
    nc = _get_program()
    shared = _host_prep(Wq, Wk, Wv, Wo, q_gamma, k_gamma, cos, sin,
                        rope_indices, mask)
    x = np.asarray(x, np.float32)
    in_maps = [dict(shared, x=np.ascontiguousarray(x[b])) for b in range(B)]
    res = run_bass_kernel_spmd(nc, in_maps, list(range(B)), trace=_trace)
    out = np.stack([res.results[b]["out"] for b in range(B)], axis=0)
    if _trace:
        return out, res
    return out


# revision 28
# speedup vs baseline: 1.0416x; 1.0416x over previous
"""Bass/Tile TRN2 kernel for nn_MultiHeadAttention_56066503082210.

Full-input contract: kernel(**inputs) takes the complete tensors and returns
the complete [B, N, D] output. Internally shards batch across 8 NeuronCores
(data parallel, no collectives) and runs one SPMD Bass program per core.

Per-core pipeline (batch b):
  x.T via PE transposes -> q/k/v projections (fp32r matmuls)
  RMSNorm + 2D RoPE on q/k in natural [n, dout] layout (gamma & rope-validity
  folded into host-precomputed cos/sin tables), then PE-transpose to [hd, n]
  S.T = k.T^T q.T per head -> tanh softcap (ACT) -> exp (ACT) -> mask multiply
  P.T @ [V | ones] accumulates attention output AND softmax denominators
  normalize, then final projection with Wo.
"""

import sys

for p in ("/opt/trn_rl_repo", "/root/.axon_site/_ro/trn_rl_repo"):
    if p not in sys.path:
        sys.path.insert(0, p)

import numpy as np
import ml_dtypes

import concourse.bass as bass
import concourse.mybir as mybir
import concourse.tile as tile
from concourse.tile import TileContext
from concourse.masks import make_identity
from concourse.bass_utils import run_bass_kernel_spmd

# ---------------------------------------------------------------- constants
B, N, D, H, HD = 8, 1024, 1024, 16, 64
NT = N // 128          # n tiles
KT = D // 128          # contraction chunks
SOFT_CAP = 50.0
EPS = 1e-6
SCALE = HD ** -0.5     # 1/8
N_ONES = 32            # replicated ones columns in V_aug (denominator rows)
VCOLS = HD + N_ONES    # 96
F32 = mybir.dt.float32
F32R = mybir.dt.float32r
BF16 = mybir.dt.bfloat16
TANH = mybir.ActivationFunctionType.Tanh
EXP = mybir.ActivationFunctionType.Exp
SQRT = mybir.ActivationFunctionType.Sqrt
ADD = mybir.AluOpType.add
MULT = mybir.AluOpType.mult

# ------------------------------------------------- walrus compat monkeypatches
# This walrus build accepts at most ONE semaphore wait per instruction for
# several instruction types (fp32r Matmult, Drain, ...). Split excess waits
# onto injected same-engine NoOps, which execute the waits in program order.
_PATCHED = False


def _apply_patches():
    global _PATCHED
    if _PATCHED:
        return
    _PATCHED = True

    _orig_lower = TileContext._lower_ordered_insts

    def _split_waits(self, ordered):
        counter = [0]
        for bb_name, insts in ordered.items():
            out = []
            for inst in insts:
                si = inst.sync_info
                waits = list(si.on_wait or []) if si is not None else []
                if len(waits) > 1:
                    for w in waits[:-1]:
                        counter[0] += 1
                        nop = mybir.InstNoOp(
                            name=f"I-waitsplit-{bb_name}-{counter[0]}",
                            engine=inst.engine,
                            ins=[],
                            outs=[],
                            sync_info=mybir.SyncInfo(on_wait=[w], on_update=[]),
                        )
                        out.append(nop)
                    si.on_wait = waits[-1:]
                out.append(inst)
            insts[:] = out
        return _orig_lower(self, ordered)

    TileContext._lower_ordered_insts = _split_waits

    def _patched_drain(self, tick_clock, wait_clock):
        nc = self.nc
        drain_inst = nc.sync.drain()
        wait_clock.add_sem_waits(
            drain_inst.ins, tile.ScopedClock({None: tick_clock.global_clock})
        )
        si = drain_inst.ins.sync_info
        waits = list(si.on_wait or []) if si is not None else []
        if len(waits) > 1:
            si.on_wait = waits[:1]
            for w in waits[1:]:
                n = nc.sync.nop(nofuse=True, hint="tail_wait_split")
                n.ins.sync_info = mybir.SyncInfo(on_wait=[w], on_update=[])
            nc.sync.drain()
        nc.all_engine_barrier()
        assert self.sems is not None
        popped = nc._tile_sem_poison_stack.pop()
        assert popped is self._sem_poison
        nc.clear_and_free_semaphores(list(self.sems.allocated().values()))
        nc.all_engine_barrier()

    TileContext._drain_and_barrier = _patched_drain

    # Enable walrus LDWEIGHTS dedup (adjacent matmuls sharing a stationary
    # operand skip the redundant reload). Correctness is revalidated by the
    # accuracy check after every change.
    import concourse.bass_utils as _bu

    _orig_run_command = _bu.run_command

    def _run_command(cmd, *a, **kw):
        if isinstance(cmd, list):
            pass  # ldw-opt stays disabled: walrus emits wrong-length ISA with it on
        return _orig_run_command(cmd, *a, **kw)

    _bu.run_command = _run_command


# ------------------------------------------------------------- device program
def build_program():
    _apply_patches()
    nc = bass.Bass()

    x_d = nc.dram_tensor("x", [N, D], F32, kind="ExternalInput")
    wq_d = nc.dram_tensor("wqT", [D, D], F32R, kind="ExternalInput")
    wk_d = nc.dram_tensor("wkT", [D, D], F32R, kind="ExternalInput")
    wv_d = nc.dram_tensor("wvT", [D, D], F32R, kind="ExternalInput")
    wo_d = nc.dram_tensor("woT", [D, D], F32R, kind="ExternalInput")
    cosq_d = nc.dram_tensor("cosq", [N, HD], F32, kind="ExternalInput")
    sinq_d = nc.dram_tensor("sinq", [N, HD], F32, kind="ExternalInput")
    cosk_d = nc.dram_tensor("cosk", [N, HD], F32, kind="ExternalInput")
    sink_d = nc.dram_tensor("sink", [N, HD], F32, kind="ExternalInput")
    mask_d = nc.dram_tensor("mask01T", [N, N], BF16, kind="ExternalInput")
    out_d = nc.dram_tensor("out", [N, D], F32, kind="ExternalOutput")

    with TileContext(nc) as tc:
        with (
            tc.tile_pool(name="pa", bufs=1) as pa,
            tc.tile_pool(name="pqk", bufs=1) as pqk,
        ):
            ident = pa.tile([128, 128], F32)
            make_identity(nc, ident[:])
            eps_b = pa.tile([128, 1], F32)
            nc.vector.memset(eps_b[:], EPS)

            # rope tables, [p, nt, j] layout
            tabs = {}
            for name, d in (("cosq", cosq_d), ("sinq", sinq_d),
                            ("cosk", cosk_d), ("sink", sink_d)):
                t = pa.tile([128, NT, HD], F32, tag=name)
                nc.sync.dma_start(t[:], d.rearrange("(t p) j -> p t j", p=128))
                tabs[name] = t

            # V_aug [p, h, c, col]: col<HD = v values, col>=HD = 1.0
            vaug = pa.tile([128, H, KT, VCOLS], F32R)
            one_c = pa.tile([128, 1], F32)
            nc.vector.memset(one_c[:], 1.0)
            nc.vector.tensor_copy(
                vaug[:, :, :, HD:VCOLS],
                one_c[:, None, None, :].broadcast_to([128, H, KT, N_ONES]))

            # persistent transposed q/k; qnT is later overwritten per-head with
            # the normalized attention output O.T (same lifetime handoff)
            qnT = pqk.tile([128, KT, N], F32R)
            knT = pqk.tile([128, KT, N], F32R)

            with (
                tc.tile_pool(name="pxs", bufs=1) as pxs,
                tc.tile_pool(name="px", bufs=2) as px,
                tc.tile_pool(name="ps_tp", bufs=2, space="PSUM") as ps_tp,
                tc.tile_pool(name="ps_mm", bufs=2, space="PSUM") as ps_mm,
            ):
                # ---- phase 0: load x, build x.T
                xT = pxs.tile([128, KT, N], F32R)
                for nt in range(NT):
                    xin = px.tile([128, D], F32, tag="xin")
                    nc.sync.dma_start(xin[:], x_d[nt * 128:(nt + 1) * 128, :])
                    for kt in range(KT):
                        tp = ps_tp.tile([128, 128], F32, tag="tp")
                        nc.tensor.transpose(tp[:], xin[:, kt * 128:(kt + 1) * 128], ident[:])
                        nc.vector.tensor_copy(xT[:, kt, nt * 128:(nt + 1) * 128], tp[:])

                # ---- phase 1: projections (whole W resident per projection)
                def load_w_all(dram):
                    w = pxs.tile([128, KT, D], F32R, tag="wall")
                    for kt in range(KT):
                        nc.sync.dma_start(
                            w[:, kt, :], dram[kt * 128:(kt + 1) * 128, :])
                    return w

                # q / k with norm + rope, written transposed
                for kind in ("q", "k"):
                    w = load_w_all(wq_d if kind == "q" else wk_d)
                    cos_t = tabs["cosq" if kind == "q" else "cosk"]
                    sin_t = tabs["sinq" if kind == "q" else "sink"]
                    dst = qnT if kind == "q" else knT
                    for nt in range(NT):
                        # both halves accumulate together so adjacent matmuls
                        # share the stationary xT chunk (LDW dedup)
                        accs = [ps_mm.tile([128, 512], F32, tag=f"acc{dh}", name=f"acc{dh}")
                                for dh in range(2)]
                        for kt in range(KT):
                            for dh in range(2):
                                nc.tensor.matmul(
                                    accs[dh][:],
                                    xT[:, kt, nt * 128:(nt + 1) * 128],
                                    w[:, kt, dh * 512:(dh + 1) * 512],
                                    start=(kt == 0), stop=(kt == KT - 1),
                                )
                        for dh in range(2):
                            acc = accs[dh]
                            # copy projection to SBUF (DVE reads 1 PSUM max)
                            qc = px.tile([128, 8, HD], F32, tag="qc")
                            nc.vector.tensor_copy(
                                qc[:], acc[:].rearrange("p (g j) -> p g j", g=8))
                            a3 = qc[:]
                            # sum of squares per 64-wide head group
                            sq = px.tile([128, 8, HD], F32, tag="sq")
                            nc.vector.tensor_mul(sq[:], a3, a3)
                            ssq = px.tile([128, 8], F32, tag="ssq")
                            nc.vector.tensor_reduce(
                                ssq[:], sq[:], axis=mybir.AxisListType.X, op=ADD)
                            rstd = px.tile([128, 8], F32, tag="rstd")
                            nc.scalar.activation(
                                rstd[:], ssq[:], SQRT, bias=eps_b[:], scale=1.0 / HD)
                            nc.vector.reciprocal(rstd[:], rstd[:])
                            # normalize in place
                            nc.vector.tensor_mul(
                                qc[:], a3,
                                rstd[:, :, None].broadcast_to([128, 8, HD]))
                            # rope (gamma and validity folded into tables)
                            qr = px.tile([128, 8, HD], F32, tag="sq")
                            tmp = px.tile([128, 8, 32], F32, tag="tmp")
                            c_lo = cos_t[:, nt, 0:32][:, None, :].broadcast_to([128, 8, 32])
                            c_hi = cos_t[:, nt, 32:64][:, None, :].broadcast_to([128, 8, 32])
                            s_lo = sin_t[:, nt, 0:32][:, None, :].broadcast_to([128, 8, 32])
                            s_hi = sin_t[:, nt, 32:64][:, None, :].broadcast_to([128, 8, 32])
                            nc.vector.tensor_mul(qr[:, :, 0:32], qc[:, :, 0:32], c_lo)
                            nc.vector.tensor_mul(tmp[:], qc[:, :, 32:64], s_lo)
                            nc.vector.tensor_add(qr[:, :, 0:32], qr[:, :, 0:32], tmp[:])
                            nc.vector.tensor_mul(qr[:, :, 32:64], qc[:, :, 32:64], c_hi)
                            nc.vector.tensor_mul(tmp[:], qc[:, :, 0:32], s_hi)
                            nc.vector.tensor_add(qr[:, :, 32:64], qr[:, :, 32:64], tmp[:])
                            # transpose 4 blocks of 128 into dst
                            qr2 = qr[:].rearrange("p g j -> p (g j)")
                            for j in range(4):
                                dt = dh * 4 + j
                                tp = ps_tp.tile([128, 128], F32, tag="tp")
                                nc.tensor.transpose(
                                    tp[:], qr2[:, j * 128:(j + 1) * 128], ident[:])
                                nc.vector.tensor_copy(
                                    dst[:, dt, nt * 128:(nt + 1) * 128], tp[:])

                # v: natural layout straight into V_aug
                w = load_w_all(wv_d)
                for nt in range(NT):
                    accs = [ps_mm.tile([128, 512], F32, tag=f"acc{dh}", name=f"acc{dh}")
                            for dh in range(2)]
                    for kt in range(KT):
                        for dh in range(2):
                            nc.tensor.matmul(
                                accs[dh][:],
                                xT[:, kt, nt * 128:(nt + 1) * 128],
                                w[:, kt, dh * 512:(dh + 1) * 512],
                                start=(kt == 0), stop=(kt == KT - 1),
                            )
                    for dh in range(2):
                        nc.vector.tensor_copy(
                            vaug[:, dh * 8:(dh + 1) * 8, nt, 0:HD],
                            accs[dh][:].rearrange("p (g j) -> p g j", g=8)[:, :, None, :],
                        )

            # ---- phase 2: attention per head
            with (
                tc.tile_pool(name="pls", bufs=1) as pls,
                tc.tile_pool(name="pl", bufs=2) as pl,
            ):
                woT = pls.tile([128, KT, D], F32R)
                for kt in range(KT):
                    nc.sync.dma_start(
                        woT[:, kt, :], wo_d[kt * 128:(kt + 1) * 128, :])

                # mask01 transposed: [p, c, nq] (bf16 0/1)
                maskm = pls.tile([128, NT, N], BF16)
                nc.sync.dma_start(
                    maskm[:], mask_d.rearrange("(c p) q -> p c q", p=128))

                with (
                    tc.tile_pool(name="ps_sa", bufs=1, space="PSUM") as ps_sa,
                    tc.tile_pool(name="ps_sb", bufs=1, space="PSUM") as ps_sb,
                    tc.tile_pool(name="ps_pv", bufs=2, space="PSUM") as ps_pv,
                ):
                    for g in range(H // 2):
                        heads = (2 * g, 2 * g + 1)
                        spool = {2 * g: ps_sa, 2 * g + 1: ps_sb}
                        pvs = {}
                        for h in heads:
                            pvs[h] = ps_pv.tile([VCOLS, N], F32, tag="pv",
                                                name=f"pv{h}")
                        for c in range(KT):
                            pms = {}
                            for h in heads:
                                r0 = 64 * (h % 2)
                                dt = h // 2
                                qh = qnT[r0:r0 + 64, dt, :]
                                kh = knT[r0:r0 + 64, dt, :]
                                s1 = spool[h].tile([128, N], F32, tag="s1",
                                                   name=f"s1_{h}_{c}")
                                for half in range(2):
                                    nc.tensor.matmul(
                                        s1[:, half * 512:(half + 1) * 512],
                                        kh[:, c * 128:(c + 1) * 128],
                                        qh[:, half * 512:(half + 1) * 512],
                                        start=True, stop=True,
                                    )
                                t = pl.tile([128, N], F32, tag="t")
                                nc.scalar.activation(t[:], s1[:], TANH,
                                                     scale=SCALE / SOFT_CAP)
                                e = pl.tile([128, N], F32, tag="e")
                                nc.scalar.activation(e[:], t[:], EXP,
                                                     scale=SOFT_CAP)
                                pm = pl.tile([128, N], F32R, tag="pm")
                                nc.gpsimd.tensor_mul(pm[:], e[:],
                                                     maskm[:, c, :])
                                pms[h] = pm
                            for h in heads:
                                for half in range(2):
                                    nc.tensor.matmul(
                                        pvs[h][:, half * 512:(half + 1) * 512],
                                        vaug[:, h, c, :],
                                        pms[h][:, half * 512:(half + 1) * 512],
                                        start=(c == 0), stop=(c == KT - 1),
                                    )
                        for h in heads:
                            r0 = 64 * (h % 2)
                            dt = h // 2
                            pv = pvs[h]
                            recip = pl.tile([N_ONES, N], F32, tag="recip")
                            nc.vector.reciprocal(recip[:], pv[HD:VCOLS, :])
                            # normalized O.T into qnT storage (q rows dead)
                            for i in range(HD // N_ONES):
                                nc.vector.tensor_mul(
                                    qnT[r0 + i * N_ONES:r0 + (i + 1) * N_ONES,
                                        dt, :],
                                    pv[i * N_ONES:(i + 1) * N_ONES, :],
                                    recip[:],
                                )

                # ---- phase 3: output projection
                with tc.tile_pool(name="ps_o", bufs=2, space="PSUM") as ps_o:
                    for nt in range(NT):
                        accs = [ps_o.tile([128, 512], F32, tag=f"oacc{dh}", name=f"oacc{dh}")
                                for dh in range(2)]
                        for kt in range(KT):
                            for dh in range(2):
                                nc.tensor.matmul(
                                    accs[dh][:],
                                    qnT[:, kt, nt * 128:(nt + 1) * 128],
                                    woT[:, kt, dh * 512:(dh + 1) * 512],
                                    start=(kt == 0), stop=(kt == KT - 1),
                                )
                        for dh in range(2):
                            osb = pl.tile([128, 512], F32, tag="osb")
                            nc.vector.tensor_copy(osb[:], accs[dh][:])
                            nc.sync.dma_start(
                                out_d[nt * 128:(nt + 1) * 128,
                                      dh * 512:(dh + 1) * 512], osb[:])
    return nc


_NC_CACHE = None


def _get_program():
    global _NC_CACHE
    if _NC_CACHE is None:
        _NC_CACHE = build_program()
    return _NC_CACHE


# ------------------------------------------------------------------ host side
def _host_prep(Wq, Wk, Wv, Wo, q_gamma, k_gamma, cos, sin, rope_indices, mask):
    f = np.float32
    wqT = np.ascontiguousarray(np.asarray(Wq, f).T)
    wkT = np.ascontiguousarray(np.asarray(Wk, f).T)
    wvT = np.ascontiguousarray(np.asarray(Wv, f).T)
    woT = np.ascontiguousarray(np.asarray(Wo, f).T)

    idx = np.asarray(rope_indices)
    valid = (idx >= 0)
    safe = np.clip(idx, 0, None).astype(np.int64)
    cos_sel = np.asarray(cos, f)[safe]          # [N, HD]
    sin_sel = np.asarray(sin, f)[safe]
    cos_eff = np.where(valid[:, None], cos_sel, f(1.0))
    sin_eff = np.where(valid[:, None], sin_sel, f(0.0))
    # rotate_half sign: -sin on first half, +sin on second
    sin_signed = np.concatenate([-sin_eff[:, :32], sin_eff[:, 32:]], axis=1)
    gq = np.asarray(q_gamma, f)
    gk = np.asarray(k_gamma, f)
    gq_swap = np.concatenate([gq[32:], gq[:32]])
    gk_swap = np.concatenate([gk[32:], gk[:32]])
    cosq = np.ascontiguousarray(cos_eff * gq[None, :])
    sinq = np.ascontiguousarray(sin_signed * gq_swap[None, :])
    cosk = np.ascontiguousarray(cos_eff * gk[None, :])
    sink = np.ascontiguousarray(sin_signed * gk_swap[None, :])

    m01T = np.ascontiguousarray(
        np.asarray(mask).astype(np.float32).T.astype(ml_dtypes.bfloat16))
    return dict(wqT=wqT, wkT=wkT, wvT=wvT, woT=woT,
                cosq=cosq, sinq=sinq, cosk=cosk, sink=sink, mask01T=m01T)


def _ensure_profile_hook():
    """Register the NTFF profile hook (missing antenv.axon_hooks shim)."""
    import types

    try:
        from antenv.axon_hooks import get_axon_ntff_profile_hook
        if get_axon_ntff_profile_hook() is not None:
            return
        import antenv.axon_hooks as mod
    except ImportError:
        import antenv
        mod = types.ModuleType("antenv.axon_hooks")
        holder = {}
        mod.set_axon_ntff_profile_hook = lambda h: holder.__setitem__("h", h)
        mod.get_axon_ntff_profile_hook = lambda: holder.get("h")
        sys.modules["antenv.axon_hooks"] = mod
        antenv.axon_hooks = mod
    if "/root/.axon_site" not in sys.path:
        sys.path.insert(0, "/root/.axon_site")
    from trn_agent_boot.trn_boot import _ntff_profile_via_ctypes
    hook = _ntff_profile_via_ctypes("/opt/axon/libaxon_pjrt.so")
    if hook is not None:
        mod.set_axon_ntff_profile_hook(hook)


def kernel(x, Wq, Wk, Wv, Wo, q_gamma, k_gamma, cos, sin, rope_indices, mask,
           _trace=False):
    if _trace:
        _ensure_profile_hook()
    nc = _get_program()
    shared = _host_prep(Wq, Wk, Wv, Wo, q_gamma, k_gamma, cos, sin,
                        rope_indices, mask)
    x = np.asarray(x, np.float32)
    in_maps = [dict(shared, x=np.ascontiguousarray(x[b])) for b in range(B)]
    res = run_bass_kernel_spmd(nc, in_maps, list(range(B)), trace=_trace)
    out = np.stack([res.results[b]["out"] for b in range(B)], axis=0)
    if _trace:
        return out, res
    return out


# revision 30
# speedup vs baseline: 1.2072x; 1.1590x over previous
"""Bass/Tile TRN2 kernel for nn_MultiHeadAttention_56066503082210.

Full-input contract: kernel(**inputs) takes the complete tensors and returns
the complete [B, N, D] output. Internally shards batch across 8 NeuronCores
(data parallel, no collectives) and runs one SPMD Bass program per core.

Per-core pipeline (batch b):
  x.T via PE transposes -> q/k/v projections (fp32r matmuls)
  RMSNorm + 2D RoPE on q/k in natural [n, dout] layout (gamma & rope-validity
  folded into host-precomputed cos/sin tables), then PE-transpose to [hd, n]
  S.T = k.T^T q.T per head -> tanh softcap (ACT) -> exp (ACT) -> mask multiply
  P.T @ [V | ones] accumulates attention output AND softmax denominators
  normalize, then final projection with Wo.
"""

import sys

for p in ("/opt/trn_rl_repo", "/root/.axon_site/_ro/trn_rl_repo"):
    if p not in sys.path:
        sys.path.insert(0, p)

import numpy as np
import ml_dtypes

import concourse.bass as bass
import concourse.mybir as mybir
import concourse.tile as tile
from concourse.tile import TileContext
from concourse.masks import make_identity
from concourse.bass_utils import run_bass_kernel_spmd

# ---------------------------------------------------------------- constants
B, N, D, H, HD = 8, 1024, 1024, 16, 64
NT = N // 128          # n tiles
KT = D // 128          # contraction chunks
SOFT_CAP = 50.0
EPS = 1e-6
SCALE = HD ** -0.5     # 1/8
N_ONES = 32            # replicated ones columns in V_aug (denominator rows)
VCOLS = HD + N_ONES    # 96
F32 = mybir.dt.float32
F32R = mybir.dt.float32r
BF16 = mybir.dt.bfloat16
TANH = mybir.ActivationFunctionType.Tanh
EXP = mybir.ActivationFunctionType.Exp
SQRT = mybir.ActivationFunctionType.Sqrt
ADD = mybir.AluOpType.add
MULT = mybir.AluOpType.mult

# ------------------------------------------------- walrus compat monkeypatches
# This walrus build accepts at most ONE semaphore wait per instruction for
# several instruction types (fp32r Matmult, Drain, ...). Split excess waits
# onto injected same-engine NoOps, which execute the waits in program order.
_PATCHED = False


def _apply_patches():
    global _PATCHED
    if _PATCHED:
        return
    _PATCHED = True

    _orig_lower = TileContext._lower_ordered_insts

    def _split_waits(self, ordered):
        counter = [0]
        for bb_name, insts in ordered.items():
            out = []
            for inst in insts:
                si = inst.sync_info
                waits = list(si.on_wait or []) if si is not None else []
                if len(waits) > 1:
                    for w in waits[:-1]:
                        counter[0] += 1
                        nop = mybir.InstNoOp(
                            name=f"I-waitsplit-{bb_name}-{counter[0]}",
                            engine=inst.engine,
                            ins=[],
                            outs=[],
                            sync_info=mybir.SyncInfo(on_wait=[w], on_update=[]),
                        )
                        out.append(nop)
                    si.on_wait = waits[-1:]
                out.append(inst)
            insts[:] = out
        return _orig_lower(self, ordered)

    TileContext._lower_ordered_insts = _split_waits

    def _patched_drain(self, tick_clock, wait_clock):
        nc = self.nc
        drain_inst = nc.sync.drain()
        wait_clock.add_sem_waits(
            drain_inst.ins, tile.ScopedClock({None: tick_clock.global_clock})
        )
        si = drain_inst.ins.sync_info
        waits = list(si.on_wait or []) if si is not None else []
        if len(waits) > 1:
            si.on_wait = waits[:1]
            for w in waits[1:]:
                n = nc.sync.nop(nofuse=True, hint="tail_wait_split")
                n.ins.sync_info = mybir.SyncInfo(on_wait=[w], on_update=[])
            nc.sync.drain()
        nc.all_engine_barrier()
        assert self.sems is not None
        popped = nc._tile_sem_poison_stack.pop()
        assert popped is self._sem_poison
        nc.clear_and_free_semaphores(list(self.sems.allocated().values()))
        nc.all_engine_barrier()

    TileContext._drain_and_barrier = _patched_drain

    # Enable walrus LDWEIGHTS dedup (adjacent matmuls sharing a stationary
    # operand skip the redundant reload). Correctness is revalidated by the
    # accuracy check after every change.
    import concourse.bass_utils as _bu

    _orig_run_command = _bu.run_command

    def _run_command(cmd, *a, **kw):
        if isinstance(cmd, list):
            pass  # ldw-opt stays disabled: walrus emits wrong-length ISA with it on
        return _orig_run_command(cmd, *a, **kw)

    _bu.run_command = _run_command


# ------------------------------------------------------------- device program
def build_program():
    _apply_patches()
    nc = bass.Bass()

    x_d = nc.dram_tensor("x", [N, D], F32, kind="ExternalInput")
    wq_d = nc.dram_tensor("wqT", [D, D], F32R, kind="ExternalInput")
    wk_d = nc.dram_tensor("wkT", [D, D], F32R, kind="ExternalInput")
    wv_d = nc.dram_tensor("wvT", [D, D], F32R, kind="ExternalInput")
    wo_d = nc.dram_tensor("woT", [D, D], F32R, kind="ExternalInput")
    cosq_d = nc.dram_tensor("cosq", [N, HD], F32, kind="ExternalInput")
    sinq_d = nc.dram_tensor("sinq", [N, HD], F32, kind="ExternalInput")
    cosk_d = nc.dram_tensor("cosk", [N, HD], F32, kind="ExternalInput")
    sink_d = nc.dram_tensor("sink", [N, HD], F32, kind="ExternalInput")
    mask_d = nc.dram_tensor("mask01T", [N, N], BF16, kind="ExternalInput")
    out_d = nc.dram_tensor("out", [N, D], F32, kind="ExternalOutput")

    with TileContext(nc) as tc:
        with (
            tc.tile_pool(name="pa", bufs=1) as pa,
            tc.tile_pool(name="pqk", bufs=1) as pqk,
        ):
            ident = pa.tile([128, 128], F32)
            make_identity(nc, ident[:])
            eps_b = pa.tile([128, 1], F32)
            nc.vector.memset(eps_b[:], EPS)

            # rope tables, [p, nt, j] layout
            tabs = {}
            for name, d in (("cosq", cosq_d), ("sinq", sinq_d),
                            ("cosk", cosk_d), ("sink", sink_d)):
                t = pa.tile([128, NT, HD], F32, tag=name)
                nc.sync.dma_start(t[:], d.rearrange("(t p) j -> p t j", p=128))
                tabs[name] = t

            # V_aug [p, h, c, col]: col<HD = v values, col>=HD = 1.0
            vaug = pa.tile([128, H, KT, VCOLS], F32R)
            one_c = pa.tile([128, 1], F32)
            nc.vector.memset(one_c[:], 1.0)
            nc.vector.tensor_copy(
                vaug[:, :, :, HD:VCOLS],
                one_c[:, None, None, :].broadcast_to([128, H, KT, N_ONES]))

            # persistent transposed q/k; qnT is later overwritten per-head with
            # the normalized attention output O.T (same lifetime handoff)
            qnT = pqk.tile([128, KT, N], F32R)
            knT = pqk.tile([128, KT, N], F32R)

            with (
                tc.tile_pool(name="pxs", bufs=1) as pxs,
                tc.tile_pool(name="px", bufs=2) as px,
                tc.tile_pool(name="ps_tp", bufs=2, space="PSUM") as ps_tp,
                tc.tile_pool(name="ps_mm", bufs=2, space="PSUM") as ps_mm,
            ):
                # ---- phase 0: load x, build x.T
                xT = pxs.tile([128, KT, N], F32R)
                for nt in range(NT):
                    xin = px.tile([128, D], F32, tag="xin")
                    nc.sync.dma_start(xin[:], x_d[nt * 128:(nt + 1) * 128, :])
                    for kt in range(KT):
                        tp = ps_tp.tile([128, 128], F32, tag="tp")
                        nc.tensor.transpose(tp[:], xin[:, kt * 128:(kt + 1) * 128], ident[:])
                        nc.vector.tensor_copy(xT[:, kt, nt * 128:(nt + 1) * 128], tp[:])

                # ---- phase 1: projections (whole W resident per projection)
                def load_w_all(dram):
                    w = pxs.tile([128, KT, D], F32R, tag="wall")
                    for kt in range(KT):
                        nc.sync.dma_start(
                            w[:, kt, :], dram[kt * 128:(kt + 1) * 128, :])
                    return w

                # q / k with norm + rope, written transposed
                for kind in ("q", "k"):
                    w = load_w_all(wq_d if kind == "q" else wk_d)
                    cos_t = tabs["cosq" if kind == "q" else "cosk"]
                    sin_t = tabs["sinq" if kind == "q" else "sink"]
                    dst = qnT if kind == "q" else knT
                    for nt in range(NT):
                        # both halves accumulate together so adjacent matmuls
                        # share the stationary xT chunk (LDW dedup)
                        accs = [ps_mm.tile([128, 512], F32, tag=f"acc{dh}", name=f"acc{dh}")
                                for dh in range(2)]
                        for kt in range(KT):
                            for dh in range(2):
                                nc.tensor.matmul(
                                    accs[dh][:],
                                    xT[:, kt, nt * 128:(nt + 1) * 128],
                                    w[:, kt, dh * 512:(dh + 1) * 512],
                                    start=(kt == 0), stop=(kt == KT - 1),
                                )
                        for dh in range(2):
                            acc = accs[dh]
                            # copy projection to SBUF (DVE reads 1 PSUM max)
                            qc = px.tile([128, 8, HD], F32, tag="qc")
                            nc.vector.tensor_copy(
                                qc[:], acc[:].rearrange("p (g j) -> p g j", g=8))
                            a3 = qc[:]
                            # sum of squares per 64-wide head group
                            sq = px.tile([128, 8, HD], F32, tag="sq")
                            nc.vector.tensor_mul(sq[:], a3, a3)
                            ssq = px.tile([128, 8], F32, tag="ssq")
                            nc.vector.tensor_reduce(
                                ssq[:], sq[:], axis=mybir.AxisListType.X, op=ADD)
                            rstd = px.tile([128, 8], F32, tag="rstd")
                            nc.scalar.activation(
                                rstd[:], ssq[:], SQRT, bias=eps_b[:], scale=1.0 / HD)
                            nc.vector.reciprocal(rstd[:], rstd[:])
                            # normalize in place
                            nc.vector.tensor_mul(
                                qc[:], a3,
                                rstd[:, :, None].broadcast_to([128, 8, HD]))
                            # rope (gamma and validity folded into tables)
                            qr = px.tile([128, 8, HD], F32, tag="sq")
                            tmp = px.tile([128, 8, 32], F32, tag="tmp")
                            c_lo = cos_t[:, nt, 0:32][:, None, :].broadcast_to([128, 8, 32])
                            c_hi = cos_t[:, nt, 32:64][:, None, :].broadcast_to([128, 8, 32])
                            s_lo = sin_t[:, nt, 0:32][:, None, :].broadcast_to([128, 8, 32])
                            s_hi = sin_t[:, nt, 32:64][:, None, :].broadcast_to([128, 8, 32])
                            nc.vector.tensor_mul(qr[:, :, 0:32], qc[:, :, 0:32], c_lo)
                            nc.vector.tensor_mul(tmp[:], qc[:, :, 32:64], s_lo)
                            nc.vector.tensor_add(qr[:, :, 0:32], qr[:, :, 0:32], tmp[:])
                            nc.vector.tensor_mul(qr[:, :, 32:64], qc[:, :, 32:64], c_hi)
                            nc.vector.tensor_mul(tmp[:], qc[:, :, 0:32], s_hi)
                            nc.vector.tensor_add(qr[:, :, 32:64], qr[:, :, 32:64], tmp[:])
                            # transpose 4 blocks of 128 into dst
                            qr2 = qr[:].rearrange("p g j -> p (g j)")
                            for j in range(4):
                                dt = dh * 4 + j
                                tp = ps_tp.tile([128, 128], F32, tag="tp")
                                nc.tensor.transpose(
                                    tp[:], qr2[:, j * 128:(j + 1) * 128], ident[:])
                                nc.vector.tensor_copy(
                                    dst[:, dt, nt * 128:(nt + 1) * 128], tp[:])

                # v: natural layout straight into V_aug
                w = load_w_all(wv_d)
                for nt in range(NT):
                    accs = [ps_mm.tile([128, 512], F32, tag=f"acc{dh}", name=f"acc{dh}")
                            for dh in range(2)]
                    for kt in range(KT):
                        for dh in range(2):
                            nc.tensor.matmul(
                                accs[dh][:],
                                xT[:, kt, nt * 128:(nt + 1) * 128],
                                w[:, kt, dh * 512:(dh + 1) * 512],
                                start=(kt == 0), stop=(kt == KT - 1),
                            )
                    for dh in range(2):
                        nc.vector.tensor_copy(
                            vaug[:, dh * 8:(dh + 1) * 8, nt, 0:HD],
                            accs[dh][:].rearrange("p (g j) -> p g j", g=8)[:, :, None, :],
                        )

            # ---- phase 2: attention per head
            with (
                tc.tile_pool(name="pls", bufs=1) as pls,
                tc.tile_pool(name="pl", bufs=2) as pl,
            ):
                woT = pls.tile([128, KT, D], F32R)
                for kt in range(KT):
                    nc.sync.dma_start(
                        woT[:, kt, :], wo_d[kt * 128:(kt + 1) * 128, :])

                # mask01 transposed: [p, c, nq] (bf16 0/1)
                maskm = pls.tile([128, NT, N], BF16)
                nc.sync.dma_start(
                    maskm[:], mask_d.rearrange("(c p) q -> p c q", p=128))

                with (
                    tc.tile_pool(name="ps_s", bufs=2, space="PSUM") as ps_s,
                    tc.tile_pool(name="ps_pv", bufs=2, space="PSUM") as ps_pv,
                    tc.tile_pool(name="pl3", bufs=3) as pl3,
                ):
                    for h in range(H):
                        r0 = 64 * (h % 2)
                        dt = h // 2
                        qh = qnT[r0:r0 + 64, dt, :]
                        kh = knT[r0:r0 + 64, dt, :]
                        pv = ps_pv.tile([VCOLS, N], F32, tag="pv")
                        for c in range(KT):
                            s1 = ps_s.tile([128, N], F32, tag="s1")
                            for half in range(2):
                                nc.tensor.matmul(
                                    s1[:, half * 512:(half + 1) * 512],
                                    kh[:, c * 128:(c + 1) * 128],
                                    qh[:, half * 512:(half + 1) * 512],
                                    start=True, stop=True,
                                )
                            t = pl.tile([128, N], F32, tag="t")
                            nc.scalar.activation(t[:], s1[:], TANH,
                                                 scale=SCALE / SOFT_CAP)
                            e = pl.tile([128, N], F32, tag="e")
                            nc.scalar.activation(e[:], t[:], EXP,
                                                 scale=SOFT_CAP)
                            pm = pl3.tile([128, N], F32R, tag="pm")
                            nc.gpsimd.tensor_mul(pm[:], e[:], maskm[:, c, :])
                            for half in range(2):
                                nc.tensor.matmul(
                                    pv[:, half * 512:(half + 1) * 512],
                                    vaug[:, h, c, :],
                                    pm[:, half * 512:(half + 1) * 512],
                                    start=(c == 0), stop=(c == KT - 1),
                                )
                        recip = pls.tile([N_ONES, N], F32, tag="recip")
                        nc.vector.reciprocal(recip[:], pv[HD:VCOLS, :])
                        # write normalized O.T into qnT storage (q rows dead)
                        for i in range(HD // N_ONES):
                            nc.vector.tensor_mul(
                                qnT[r0 + i * N_ONES:r0 + (i + 1) * N_ONES,
                                    dt, :],
                                pv[i * N_ONES:(i + 1) * N_ONES, :],
                                recip[:],
                            )

                # ---- phase 3: output projection
                with tc.tile_pool(name="ps_o", bufs=2, space="PSUM") as ps_o:
                    for nt in range(NT):
                        accs = [ps_o.tile([128, 512], F32, tag=f"oacc{dh}", name=f"oacc{dh}")
                                for dh in range(2)]
                        for kt in range(KT):
                            for dh in range(2):
                                nc.tensor.matmul(
                                    accs[dh][:],
                                    qnT[:, kt, nt * 128:(nt + 1) * 128],
                                    woT[:, kt, dh * 512:(dh + 1) * 512],
                                    start=(kt == 0), stop=(kt == KT - 1),
                                )
                        for dh in range(2):
                            osb = pl.tile([128, 512], F32, tag="osb")
                            nc.vector.tensor_copy(osb[:], accs[dh][:])
                            nc.sync.dma_start(
                                out_d[nt * 128:(nt + 1) * 128,
                                      dh * 512:(dh + 1) * 512], osb[:])
    return nc


_NC_CACHE = None


def _get_program():
    global _NC_CACHE
    if _NC_CACHE is None:
        _NC_CACHE = build_program()
    return _NC_CACHE


# ------------------------------------------------------------------ host side
def _host_prep(Wq, Wk, Wv, Wo, q_gamma, k_gamma, cos, sin, rope_indices, mask):
    f = np.float32
    wqT = np.ascontiguousarray(np.asarray(Wq, f).T)
    wkT = np.ascontiguousarray(np.asarray(Wk, f).T)
    wvT = np.ascontiguousarray(np.asarray(Wv, f).T)
    woT = np.ascontiguousarray(np.asarray(Wo, f).T)

    idx = np.asarray(rope_indices)
    valid = (idx >= 0)
    safe = np.clip(idx, 0, None).astype(np.int64)
    cos_sel = np.asarray(cos, f)[safe]          # [N, HD]
    sin_sel = np.asarray(sin, f)[safe]
    cos_eff = np.where(valid[:, None], cos_sel, f(1.0))
    sin_eff = np.where(valid[:, None], sin_sel, f(0.0))
    # rotate_half sign: -sin on first half, +sin on second
    sin_signed = np.concatenate([-sin_eff[:, :32], sin_eff[:, 32:]], axis=1)
    gq = np.asarray(q_gamma, f)
    gk = np.asarray(k_gamma, f)
    gq_swap = np.concatenate([gq[32:], gq[:32]])
    gk_swap = np.concatenate([gk[32:], gk[:32]])
    cosq = np.ascontiguousarray(cos_eff * gq[None, :])
    sinq = np.ascontiguousarray(sin_signed * gq_swap[None, :])
    cosk = np.ascontiguousarray(cos_eff * gk[None, :])
    sink = np.ascontiguousarray(sin_signed * gk_swap[None, :])

    m01T = np.ascontiguousarray(
        np.asarray(mask).astype(np.float32).T.astype(ml_dtypes.bfloat16))
    return dict(wqT=wqT, wkT=wkT, wvT=wvT, woT=woT,
                cosq=cosq, sinq=sinq, cosk=cosk, sink=sink, mask01T=m01T)


def _ensure_profile_hook():
    """Register the NTFF profile hook (missing antenv.axon_hooks shim)."""
    import types

    try:
        from antenv.axon_hooks import get_axon_ntff_profile_hook
        if get_axon_ntff_profile_hook() is not None:
            return
        import antenv.axon_hooks as mod
    except ImportError:
        import antenv
        mod = types.ModuleType("antenv.axon_hooks")
        holder = {}
        mod.set_axon_ntff_profile_hook = lambda h: holder.__setitem__("h", h)
        mod.get_axon_ntff_profile_hook = lambda: holder.get("h")
        sys.modules["antenv.axon_hooks"] = mod
        antenv.axon_hooks = mod
    if "/root/.axon_site" not in sys.path:
        sys.path.insert(0, "/root/.axon_site")
    from trn_agent_boot.trn_boot import _ntff_profile_via_ctypes
    hook = _ntff_profile_via_ctypes("/opt/axon/libaxon_pjrt.so")
    if hook is not None:
        mod.set_axon_ntff_profile_hook(hook)


def kernel(x, Wq, Wk, Wv, Wo, q_gamma, k_gamma, cos, sin, rope_indices, mask,
           _trace=False):
    if _trace:
        _ensure_profile_hook()
    nc = _get_program()
    shared = _host_prep(Wq, Wk, Wv, Wo, q_gamma, k_gamma, cos, sin,
                        rope_indices, mask)
    x = np.asarray(x, np.float32)
    in_maps = [dict(shared, x=np.ascontiguousarray(x[b])) for b in range(B)]
    res = run_bass_kernel_spmd(nc, in_maps, list(range(B)), trace=_trace)
    out = np.stack([res.results[b]["out"] for b in range(B)], axis=0)
    if _trace:
        return out, res
    return out


# revision 32
# speedup vs baseline: 1.2081x; 1.0007x over previous
"""Bass/Tile TRN2 kernel for nn_MultiHeadAttention_56066503082210.

Full-input contract: kernel(**inputs) takes the complete tensors and returns
the complete [B, N, D] output. Internally shards batch across 8 NeuronCores
(data parallel, no collectives) and runs one SPMD Bass program per core.

Per-core pipeline (batch b):
  x.T via PE transposes -> q/k/v projections (fp32r matmuls)
  RMSNorm + 2D RoPE on q/k in natural [n, dout] layout (gamma & rope-validity
  folded into host-precomputed cos/sin tables), then PE-transpose to [hd, n]
  S.T = k.T^T q.T per head -> tanh softcap (ACT) -> exp (ACT) -> mask multiply
  P.T @ [V | ones] accumulates attention output AND softmax denominators
  normalize, then final projection with Wo.
"""

import sys

for p in ("/opt/trn_rl_repo", "/root/.axon_site/_ro/trn_rl_repo"):
    if p not in sys.path:
        sys.path.insert(0, p)

import numpy as np
import ml_dtypes

import concourse.bass as bass
import concourse.mybir as mybir
import concourse.tile as tile
from concourse.tile import TileContext
from concourse.masks import make_identity
from concourse.bass_utils import run_bass_kernel_spmd

# ---------------------------------------------------------------- constants
B, N, D, H, HD = 8, 1024, 1024, 16, 64
NT = N // 128          # n tiles
KT = D // 128          # contraction chunks
SOFT_CAP = 50.0
EPS = 1e-6
SCALE = HD ** -0.5     # 1/8
N_ONES = 32            # replicated ones columns in V_aug (denominator rows)
VCOLS = HD + N_ONES    # 96
F32 = mybir.dt.float32
F32R = mybir.dt.float32r
BF16 = mybir.dt.bfloat16
TANH = mybir.ActivationFunctionType.Tanh
EXP = mybir.ActivationFunctionType.Exp
SQRT = mybir.ActivationFunctionType.Sqrt
ADD = mybir.AluOpType.add
MULT = mybir.AluOpType.mult

# ------------------------------------------------- walrus compat monkeypatches
# This walrus build accepts at most ONE semaphore wait per instruction for
# several instruction types (fp32r Matmult, Drain, ...). Split excess waits
# onto injected same-engine NoOps, which execute the waits in program order.
_PATCHED = False


def _apply_patches():
    global _PATCHED
    if _PATCHED:
        return
    _PATCHED = True

    _orig_lower = TileContext._lower_ordered_insts

    def _split_waits(self, ordered):
        counter = [0]
        for bb_name, insts in ordered.items():
            out = []
            for inst in insts:
                si = inst.sync_info
                waits = list(si.on_wait or []) if si is not None else []
                if len(waits) > 1:
                    for w in waits[:-1]:
                        counter[0] += 1
                        nop = mybir.InstNoOp(
                            name=f"I-waitsplit-{bb_name}-{counter[0]}",
                            engine=inst.engine,
                            ins=[],
                            outs=[],
                            sync_info=mybir.SyncInfo(on_wait=[w], on_update=[]),
                        )
                        out.append(nop)
                    si.on_wait = waits[-1:]
                out.append(inst)
            insts[:] = out
        return _orig_lower(self, ordered)

    TileContext._lower_ordered_insts = _split_waits

    def _patched_drain(self, tick_clock, wait_clock):
        nc = self.nc
        drain_inst = nc.sync.drain()
        wait_clock.add_sem_waits(
            drain_inst.ins, tile.ScopedClock({None: tick_clock.global_clock})
        )
        si = drain_inst.ins.sync_info
        waits = list(si.on_wait or []) if si is not None else []
        if len(waits) > 1:
            si.on_wait = waits[:1]
            for w in waits[1:]:
                n = nc.sync.nop(nofuse=True, hint="tail_wait_split")
                n.ins.sync_info = mybir.SyncInfo(on_wait=[w], on_update=[])
            nc.sync.drain()
        nc.all_engine_barrier()
        assert self.sems is not None
        popped = nc._tile_sem_poison_stack.pop()
        assert popped is self._sem_poison
        nc.clear_and_free_semaphores(list(self.sems.allocated().values()))
        nc.all_engine_barrier()

    TileContext._drain_and_barrier = _patched_drain

    # Enable walrus LDWEIGHTS dedup (adjacent matmuls sharing a stationary
    # operand skip the redundant reload). Correctness is revalidated by the
    # accuracy check after every change.
    import concourse.bass_utils as _bu

    _orig_run_command = _bu.run_command

    def _run_command(cmd, *a, **kw):
        if isinstance(cmd, list):
            pass  # ldw-opt stays disabled: walrus emits wrong-length ISA with it on
        return _orig_run_command(cmd, *a, **kw)

    _bu.run_command = _run_command


# ------------------------------------------------------------- device program
def build_program():
    _apply_patches()
    nc = bass.Bass()

    x_d = nc.dram_tensor("x", [N, D], F32, kind="ExternalInput")
    wq_d = nc.dram_tensor("wqT", [D, D], F32R, kind="ExternalInput")
    wk_d = nc.dram_tensor("wkT", [D, D], F32R, kind="ExternalInput")
    wv_d = nc.dram_tensor("wvT", [D, D], F32R, kind="ExternalInput")
    wo_d = nc.dram_tensor("woT", [D, D], F32R, kind="ExternalInput")
    cosq_d = nc.dram_tensor("cosq", [N, HD], F32, kind="ExternalInput")
    sinq_d = nc.dram_tensor("sinq", [N, HD], F32, kind="ExternalInput")
    cosk_d = nc.dram_tensor("cosk", [N, HD], F32, kind="ExternalInput")
    sink_d = nc.dram_tensor("sink", [N, HD], F32, kind="ExternalInput")
    mask_d = nc.dram_tensor("mask01T", [N, N], BF16, kind="ExternalInput")
    out_d = nc.dram_tensor("out", [N, D], F32, kind="ExternalOutput")

    with TileContext(nc) as tc:
        with (
            tc.tile_pool(name="pa", bufs=1) as pa,
            tc.tile_pool(name="pqk", bufs=1) as pqk,
        ):
            ident = pa.tile([128, 128], F32)
            make_identity(nc, ident[:])
            eps_b = pa.tile([128, 1], F32)
            nc.vector.memset(eps_b[:], EPS)

            # rope tables, [p, nt, j] layout
            tabs = {}
            for name, d in (("cosq", cosq_d), ("sinq", sinq_d),
                            ("cosk", cosk_d), ("sink", sink_d)):
                t = pa.tile([128, NT, HD], F32, tag=name)
                nc.sync.dma_start(t[:], d.rearrange("(t p) j -> p t j", p=128))
                tabs[name] = t

            # V_aug [p, h, c, col]: col<HD = v values, col>=HD = 1.0
            vaug = pa.tile([128, H, KT, VCOLS], F32R)
            one_c = pa.tile([128, 1], F32)
            nc.vector.memset(one_c[:], 1.0)
            nc.vector.tensor_copy(
                vaug[:, :, :, HD:VCOLS],
                one_c[:, None, None, :].broadcast_to([128, H, KT, N_ONES]))

            # persistent transposed q/k; qnT is later overwritten per-head with
            # the normalized attention output O.T (same lifetime handoff)
            qnT = pqk.tile([128, KT, N], F32R)
            knT = pqk.tile([128, KT, N], F32R)

            with (
                tc.tile_pool(name="pxs", bufs=1) as pxs,
                tc.tile_pool(name="px", bufs=2) as px,
                tc.tile_pool(name="ps_tp", bufs=2, space="PSUM") as ps_tp,
                tc.tile_pool(name="ps_mm", bufs=2, space="PSUM") as ps_mm,
            ):
                # ---- phase 0: load x, build x.T
                xT = pxs.tile([128, KT, N], F32R)
                for nt in range(NT):
                    xin = px.tile([128, D], F32, tag="xin")
                    nc.sync.dma_start(xin[:], x_d[nt * 128:(nt + 1) * 128, :])
                    for kt in range(KT):
                        tp = ps_tp.tile([128, 128], F32, tag="tp")
                        nc.tensor.transpose(tp[:], xin[:, kt * 128:(kt + 1) * 128], ident[:])
                        nc.vector.tensor_copy(xT[:, kt, nt * 128:(nt + 1) * 128], tp[:])

                # ---- phase 1: projections (whole W resident per projection)
                def load_w_all(dram):
                    w = pxs.tile([128, KT, D], F32R, tag="wall")
                    for kt in range(KT):
                        nc.sync.dma_start(
                            w[:, kt, :], dram[kt * 128:(kt + 1) * 128, :])
                    return w

                # q / k with norm + rope, written transposed
                for kind in ("q", "k"):
                    w = load_w_all(wq_d if kind == "q" else wk_d)
                    cos_t = tabs["cosq" if kind == "q" else "cosk"]
                    sin_t = tabs["sinq" if kind == "q" else "sink"]
                    dst = qnT if kind == "q" else knT
                    for nt in range(NT):
                        # both halves accumulate together so adjacent matmuls
                        # share the stationary xT chunk (LDW dedup)
                        accs = [ps_mm.tile([128, 512], F32, tag=f"acc{dh}", name=f"acc{dh}")
                                for dh in range(2)]
                        for kt in range(KT):
                            for dh in range(2):
                                nc.tensor.matmul(
                                    accs[dh][:],
                                    xT[:, kt, nt * 128:(nt + 1) * 128],
                                    w[:, kt, dh * 512:(dh + 1) * 512],
                                    start=(kt == 0), stop=(kt == KT - 1),
                                )
                        for dh in range(2):
                            acc = accs[dh]
                            # copy projection to SBUF (DVE reads 1 PSUM max)
                            qc = px.tile([128, 8, HD], F32, tag="qc")
                            nc.vector.tensor_copy(
                                qc[:], acc[:].rearrange("p (g j) -> p g j", g=8))
                            a3 = qc[:]
                            # sum of squares per 64-wide head group
                            sq = px.tile([128, 8, HD], F32, tag="sq")
                            nc.vector.tensor_mul(sq[:], a3, a3)
                            ssq = px.tile([128, 8], F32, tag="ssq")
                            nc.vector.tensor_reduce(
                                ssq[:], sq[:], axis=mybir.AxisListType.X, op=ADD)
                            rstd = px.tile([128, 8], F32, tag="rstd")
                            nc.scalar.activation(
                                rstd[:], ssq[:], SQRT, bias=eps_b[:], scale=1.0 / HD)
                            nc.vector.reciprocal(rstd[:], rstd[:])
                            # normalize in place
                            nc.vector.tensor_mul(
                                qc[:], a3,
                                rstd[:, :, None].broadcast_to([128, 8, HD]))
                            # rope (gamma and validity folded into tables)
                            qr = px.tile([128, 8, HD], F32, tag="sq")
                            tmp = px.tile([128, 8, 32], F32, tag="tmp")
                            c_lo = cos_t[:, nt, 0:32][:, None, :].broadcast_to([128, 8, 32])
                            c_hi = cos_t[:, nt, 32:64][:, None, :].broadcast_to([128, 8, 32])
                            s_lo = sin_t[:, nt, 0:32][:, None, :].broadcast_to([128, 8, 32])
                            s_hi = sin_t[:, nt, 32:64][:, None, :].broadcast_to([128, 8, 32])
                            nc.vector.tensor_mul(qr[:, :, 0:32], qc[:, :, 0:32], c_lo)
                            nc.vector.tensor_mul(tmp[:], qc[:, :, 32:64], s_lo)
                            nc.vector.tensor_add(qr[:, :, 0:32], qr[:, :, 0:32], tmp[:])
                            nc.vector.tensor_mul(qr[:, :, 32:64], qc[:, :, 32:64], c_hi)
                            nc.vector.tensor_mul(tmp[:], qc[:, :, 0:32], s_hi)
                            nc.vector.tensor_add(qr[:, :, 32:64], qr[:, :, 32:64], tmp[:])
                            # transpose 4 blocks of 128 into dst
                            qr2 = qr[:].rearrange("p g j -> p (g j)")
                            for j in range(4):
                                dt = dh * 4 + j
                                tp = ps_tp.tile([128, 128], F32, tag="tp")
                                nc.tensor.transpose(
                                    tp[:], qr2[:, j * 128:(j + 1) * 128], ident[:])
                                nc.vector.tensor_copy(
                                    dst[:, dt, nt * 128:(nt + 1) * 128], tp[:])

                # v: natural layout straight into V_aug
                w = load_w_all(wv_d)
                for nt in range(NT):
                    accs = [ps_mm.tile([128, 512], F32, tag=f"acc{dh}", name=f"acc{dh}")
                            for dh in range(2)]
                    for kt in range(KT):
                        for dh in range(2):
                            nc.tensor.matmul(
                                accs[dh][:],
                                xT[:, kt, nt * 128:(nt + 1) * 128],
                                w[:, kt, dh * 512:(dh + 1) * 512],
                                start=(kt == 0), stop=(kt == KT - 1),
                            )
                    for dh in range(2):
                        nc.vector.tensor_copy(
                            vaug[:, dh * 8:(dh + 1) * 8, nt, 0:HD],
                            accs[dh][:].rearrange("p (g j) -> p g j", g=8)[:, :, None, :],
                        )

            # ---- phase 2: attention per head
            with (
                tc.tile_pool(name="pls", bufs=1) as pls,
                tc.tile_pool(name="pl", bufs=2) as pl,
            ):
                woT = pls.tile([128, KT, D], F32R)
                for kt in range(KT):
                    nc.sync.dma_start(
                        woT[:, kt, :], wo_d[kt * 128:(kt + 1) * 128, :])

                # mask01 transposed: [p, c, nq] (bf16 0/1)
                maskm = pls.tile([128, NT, N], BF16)
                nc.sync.dma_start(
                    maskm[:], mask_d.rearrange("(c p) q -> p c q", p=128))

                with (
                    tc.tile_pool(name="ps_s", bufs=2, space="PSUM") as ps_s,
                    tc.tile_pool(name="ps_pv", bufs=2, space="PSUM") as ps_pv,
                    tc.tile_pool(name="pl3", bufs=3) as pl3,
                ):
                    for h in range(H):
                        r0 = 64 * (h % 2)
                        dt = h // 2
                        qh = qnT[r0:r0 + 64, dt, :]
                        kh = knT[r0:r0 + 64, dt, :]
                        pv = ps_pv.tile([VCOLS, N], F32, tag="pv")
                        for c in range(KT):
                            s1 = ps_s.tile([128, N], F32, tag="s1")
                            for half in range(2):
                                nc.tensor.matmul(
                                    s1[:, half * 512:(half + 1) * 512],
                                    kh[:, c * 128:(c + 1) * 128],
                                    qh[:, half * 512:(half + 1) * 512],
                                    start=True, stop=True,
                                )
                            t = pl.tile([128, N], F32, tag="t")
                            nc.scalar.activation(t[:], s1[:], TANH,
                                                 scale=SCALE / SOFT_CAP)
                            e = pl.tile([128, N], F32, tag="e")
                            nc.scalar.activation(e[:], t[:], EXP,
                                                 scale=SOFT_CAP)
                            pm = pl3.tile([128, N], F32R, tag="pm")
                            nc.gpsimd.tensor_mul(pm[:], e[:], maskm[:, c, :])
                            for half in range(2):
                                nc.tensor.matmul(
                                    pv[:, half * 512:(half + 1) * 512],
                                    vaug[:, h, c, :],
                                    pm[:, half * 512:(half + 1) * 512],
                                    start=(c == 0), stop=(c == KT - 1),
                                )
                        recip = pls.tile([N_ONES, N], F32, tag="recip")
                        nc.vector.reciprocal(recip[:], pv[HD:VCOLS, :])
                        # write normalized O.T into qnT storage (q rows dead)
                        for i in range(HD // N_ONES):
                            nc.vector.tensor_mul(
                                qnT[r0 + i * N_ONES:r0 + (i + 1) * N_ONES,
                                    dt, :],
                                pv[i * N_ONES:(i + 1) * N_ONES, :],
                                recip[:],
                            )

                # ---- phase 3: output projection
                with tc.tile_pool(name="ps_o", bufs=2, space="PSUM") as ps_o:
                    for nt in range(NT):
                        accs = [ps_o.tile([128, 512], F32, tag=f"oacc{dh}", name=f"oacc{dh}")
                                for dh in range(2)]
                        for kt in range(KT):
                            for dh in range(2):
                                nc.tensor.matmul(
                                    accs[dh][:],
                                    qnT[:, kt, nt * 128:(nt + 1) * 128],
                                    woT[:, kt, dh * 512:(dh + 1) * 512],
                                    start=(kt == 0), stop=(kt == KT - 1),
                                )
                        for dh in range(2):
                            osb = pl.tile([128, 512], F32, tag="osb")
                            nc.vector.tensor_copy(osb[:], accs[dh][:])
                            nc.sync.dma_start(
                                out_d[nt * 128:(nt + 1) * 128,
                                      dh * 512:(dh + 1) * 512], osb[:])
    return nc


_NC_CACHE = None


def _get_program():
    global _NC_CACHE
    if _NC_CACHE is None:
        _NC_CACHE = build_program()
    return _NC_CACHE


# ------------------------------------------------------------------ host side
def _host_prep(Wq, Wk, Wv, Wo, q_gamma, k_gamma, cos, sin, rope_indices, mask):
    f = np.float32
    wqT = np.ascontiguousarray(np.asarray(Wq, f).T)
    wkT = np.ascontiguousarray(np.asarray(Wk, f).T)
    wvT = np.ascontiguousarray(np.asarray(Wv, f).T)
    woT = np.ascontiguousarray(np.asarray(Wo, f).T)

    idx = np.asarray(rope_indices)
    valid = (idx >= 0)
    safe = np.clip(idx, 0, None).astype(np.int64)
    cos_sel = np.asarray(cos, f)[safe]          # [N, HD]
    sin_sel = np.asarray(sin, f)[safe]
    cos_eff = np.where(valid[:, None], cos_sel, f(1.0))
    sin_eff = np.where(valid[:, None], sin_sel, f(0.0))
    # rotate_half sign: -sin on first half, +sin on second
    sin_signed = np.concatenate([-sin_eff[:, :32], sin_eff[:, 32:]], axis=1)
    gq = np.asarray(q_gamma, f)
    gk = np.asarray(k_gamma, f)
    gq_swap = np.concatenate([gq[32:], gq[:32]])
    gk_swap = np.concatenate([gk[32:], gk[:32]])
    cosq = np.ascontiguousarray(cos_eff * gq[None, :])
    sinq = np.ascontiguousarray(sin_signed * gq_swap[None, :])
    cosk = np.ascontiguousarray(cos_eff * gk[None, :])
    sink = np.ascontiguousarray(sin_signed * gk_swap[None, :])

    m01T = np.ascontiguousarray(
        np.asarray(mask).astype(np.float32).T.astype(ml_dtypes.bfloat16))
    return dict(wqT=wqT, wkT=wkT, wvT=wvT, woT=woT,
                cosq=cosq, sinq=sinq, cosk=cosk, sink=sink, mask01T=m01T)


def _ensure_profile_hook():
    """Register the NTFF profile hook (missing antenv.axon_hooks shim)."""
    import types

    try:
        from antenv.axon_hooks import get_axon_ntff_profile_hook
        if get_axon_ntff_profile_hook() is not None:
            return
        import antenv.axon_hooks as mod
    except ImportError:
        import antenv
        mod = types.ModuleType("antenv.axon_hooks")
        holder = {}
        mod.set_axon_ntff_profile_hook = lambda h: holder.__setitem__("h", h)
        mod.get_axon_ntff_profile_hook = lambda: holder.get("h")
        sys.modules["antenv.axon_hooks"] = mod
        antenv.axon_hooks = mod
    if "/root/.axon_site" not in sys.path:
        sys.path.insert(0, "/root/.axon_site")
    from trn_agent_boot.trn_boot import _ntff_profile_via_ctypes
    hook = _ntff_profile_via_ctypes("/opt/axon/libaxon_pjrt.so")
    if hook is not None:
        mod.set_axon_ntff_profile_hook(hook)


def kernel(x, Wq, Wk, Wv, Wo, q_gamma, k_gamma, cos, sin, rope_indices, mask,
           _trace=False):
    if _trace:
        _ensure_profile_hook()
    nc = _get_program()
    shared = _host_prep(Wq, Wk, Wv, Wo, q_gamma, k_gamma, cos, sin,
                        rope_indices, mask)
    x = np.asarray(x, np.float32)
    in_maps = [dict(shared, x=np.ascontiguousarray(x[b])) for b in range(B)]
    res = run_bass_kernel_spmd(nc, in_maps, list(range(B)), trace=_trace)
    out = np.stack([res.results[b]["out"] for b in range(B)], axis=0)
    if _trace:
        return out, res
    return out
